# revision 2
# baseline (speedup 1.0000x reference)
"""v4: age-scan CVLoss kernel.

Key identity: with age_t = distance since last spike (0 at a spike) and
M_t = t - age_t (last-spike position, chained across chunks),
  sum of ISI^2 (incl. phantom 0->f gap) = l^2 - 2*(S_M - (F-l+1)*l)
  where S_M = F(F+1)/2 - sum(age).
So the device only needs, per row: per-chunk sums of y=1-x (-> k), sums of
age, the final age (-> l), and a tiny reverse scan on chunks 0/1 (-> f).

Per chunk (W=2000):
  ACT : y = Copy(-x+1)  f32->i16, accum -> sum(y)   [k_c = W - sum]
  DVE : age = ttscan(y, y, op0=mult, op1=add, initial=prev)  (chained)
  ACT : Copy(age) accum -> sum(age)
  DVE : chunks 0,1 only: reverse ttscan -> r1 (distance to first spike)
Host: stitch 2 halves per neuron, compute CV and nan-mean loss.
"""

import numpy as np

B, T, N = 16, 2000, 512
L = B * T
NCORES = 8
NPC = N // NCORES
HALVES = 2
P = NPC * HALVES
F = L // HALVES
W = 2000
NCH = F // W
NACC = 2 * NCH + 3  # ky[8] | sage[8] | age_last | r1_c0 | r1_c1

_BUILD_CACHE = {}


def build_bass(F_=F, W_=W, P_=P):
    import concourse.bass as bass
    from concourse import bacc
    import concourse.mybir as mybir
    from concourse import tile

    nch = F_ // W_
    Alu = mybir.AluOpType
    AF = mybir.ActivationFunctionType
    f32 = mybir.dt.float32
    i16 = mybir.dt.int16
    bf16 = mybir.dt.bfloat16

    nc = bacc.Bacc(trn_type="TRN2")
    x = nc.dram_tensor("x", (P_, F_), f32, kind="ExternalInput")
    acc = nc.dram_tensor("acc", (P_, NACC), f32, kind="ExternalOutput")

    with tile.TileContext(nc) as tc:
        with tc.tile_pool(name="persist", bufs=1) as pp, \
             tc.tile_pool(name="work", bufs=3) as wp:
            accs = pp.tile([P_, NACC], f32)

            def load_pass1(c):
                lo = c * W_
                xc = wp.tile([P_, W_], f32, tag="xc", name=f"xc{c}")
                nc.sync.dma_start(out=xc[:], in_=x[:, lo:lo + W_])
                yc = wp.tile([P_, W_], i16, tag="y", name=f"y{c}")
                nc.scalar.activation(
                    out=yc[:], in_=xc[:], func=AF.Copy,
                    scale=-1.0, bias=1.0,
                    accum_out=accs[:, c:c + 1])
                return yc

            def scan(c, yc, prev_age):
                age_c = wp.tile([P_, W_], i16, tag="age", name=f"age{c}")
                init = 0.0 if prev_age is None else prev_age[:, W_ - 1:W_]
                nc.vector.tensor_tensor_scan(
                    out=age_c[:], data0=yc[:], data1=yc[:], initial=init,
                    op0=Alu.mult, op1=Alu.add)
                return age_c

            def rev_scan(c, yc):
                rev = wp.tile([P_, W_], i16, tag="rev", name=f"rev{c}")
                nc.vector.tensor_tensor_scan(
                    out=rev[:, ::-1], data0=yc[:, ::-1], data1=yc[:, ::-1],
                    initial=0.0, op0=Alu.mult, op1=Alu.add)
                nc.vector.tensor_scalar(
                    out=accs[:, 2 * nch + 1 + c:2 * nch + 2 + c],
                    in0=rev[:, 0:1], scalar1=0.0, scalar2=None, op0=Alu.add)

            def sum_age(c, age_c):
                dummy = wp.tile([P_, W_], bf16, tag="dummy", name=f"dm{c}")
                nc.scalar.activation(
                    out=dummy[:], in_=age_c[:], func=AF.Copy,
                    accum_out=accs[:, nch + c:nch + c + 1])

            prev_age = None
            pending = None
            for c in range(nch):
                yc = load_pass1(c)
                if pending is not None:
                    sum_age(*pending)
                age_c = scan(c, yc, prev_age)
                if c < 2:
                    rev_scan(c, yc)
                pending = (c, age_c)
                prev_age = age_c
            sum_age(*pending)
            nc.vector.tensor_scalar(
                out=accs[:, 2 * nch:2 * nch + 1],
                in0=prev_age[:, W_ - 1:W_], scalar1=0.0, scalar2=None,
                op0=Alu.add)

            nc.sync.dma_start(out=acc[:], in_=accs[:])
    nc.finalize()
    return nc


def get_bass():
    key = (F, W, P)
    if key not in _BUILD_CACHE:
        _BUILD_CACHE[key] = build_bass()
    return _BUILD_CACHE[key]


def shard_input(output_spikes):
    x = np.asarray(output_spikes, dtype=np.float32)
    maps = []
    for c in range(NCORES):
        xc = x[:, :, c * NPC:(c + 1) * NPC]
        xt = np.ascontiguousarray(np.transpose(xc, (2, 0, 1))).reshape(NPC, L)
        maps.append({"x": xt.reshape(P, F)})
    return maps


def finish_host(acc_list, target_cv, F_=F, W_=W, nch=NCH):
    """Merge per-row device stats into the scalar loss (float64)."""
    target = np.asarray(target_cv, dtype=np.float64)
    sq_sum = 0.0
    n_valid = 0
    for ci, a in enumerate(acc_list):
        a = np.asarray(a, dtype=np.float64)
        ky = a[:, 0:nch]
        sage = a[:, nch:2 * nch]
        age_last = a[:, 2 * nch]
        r1 = a[:, 2 * nch + 1:2 * nch + 3]
        k_c = W_ - ky
        k = k_c.sum(axis=1)
        S_age = sage.sum(axis=1)
        ll = F_ - age_last
        S_M = F_ * (F_ + 1) / 2.0 - S_age
        s2_ph = ll * ll - 2.0 * (S_M - (F_ - ll + 1.0) * ll)
        f = np.where(k_c[:, 0] > 0, r1[:, 0] + 1.0,
                     np.where(k_c[:, 1] > 0, W_ + r1[:, 1] + 1.0, 1.0))
        s2 = s2_ph - f * f
        n_neu = a.shape[0] // 2
        for n in range(n_neu):
            p0, p1 = 2 * n, 2 * n + 1
            k0, k1 = k[p0], k[p1]
            kt = k0 + k1
            if kt < 3:
                continue
            if k0 > 0 and k1 > 0:
                s2t = s2[p0] + s2[p1] + (F_ + f[p1] - ll[p0]) ** 2
                s1t = F_ + ll[p1] - f[p0]
            elif k0 > 0:
                s2t, s1t = s2[p0], ll[p0] - f[p0]
            else:
                s2t, s1t = s2[p1], ll[p1] - f[p1]
            mean = s1t / (kt - 1.0)
            var = (s2t - s1t * s1t / (kt - 1.0)) / (kt - 2.0)
            std = np.sqrt(var) if var > 0 else 0.0
            if mean <= 0:
                continue
            cv = std / max(mean, 1e-12)
            d = cv - target[ci * NPC + n]
            sq_sum += d * d
            n_valid += 1
    return np.float32(sq_sum / max(n_valid, 1))


def ensure_ntff_hook(so_path="/opt/axon/libaxon_pjrt.so"):
    """Shim antenv.axon_hooks (absent in this image) so trace=True works.

    Mirrors trn_boot._ntff_profile_via_ctypes: drives NRT profiling via the
    axon PJRT .so's C ABI. Safe no-op if anything is missing.
    """
    import sys
    try:
        import antenv.axon_hooks  # noqa: F401
        return
    except ImportError:
        pass
    try:
        import ctypes
        import contextlib
        import types
        import os

        if not os.path.exists(so_path):
            return
        lib = ctypes.CDLL(so_path)
        if not hasattr(lib, "axon_start_nrt_profile"):
            return
        lib.axon_start_nrt_profile.argtypes = [
            ctypes.POINTER(ctypes.c_int64), ctypes.c_size_t]
        lib.axon_start_nrt_profile.restype = ctypes.c_int64
        lib.axon_stop_nrt_profile.argtypes = [ctypes.c_char_p]
        lib.axon_stop_nrt_profile.restype = ctypes.c_int64

        @contextlib.contextmanager
        def _hook(output_dir, device_ids):
            import jax
            jax.devices()
            if device_ids:
                ids = (ctypes.c_int64 * len(device_ids))(*device_ids)
                rc = lib.axon_start_nrt_profile(ids, len(device_ids))
            else:
                rc = lib.axon_start_nrt_profile(None, 0)
            if rc != 0:
                raise RuntimeError(f"axon_start_nrt_profile rc={rc}")
            try:
                yield
            finally:
                n = lib.axon_stop_nrt_profile(str(output_dir).encode())
                print(f"profile: {n} file(s) written to {output_dir}",
                      file=sys.stderr)

        mod = types.ModuleType("antenv.axon_hooks")
        mod.get_axon_ntff_profile_hook = lambda: _hook
        mod.set_axon_ntff_profile_hook = lambda h: None
        import antenv
        sys.modules["antenv.axon_hooks"] = mod
        antenv.axon_hooks = mod
    except Exception:
        pass


def kernel(output_spikes, target_cv):
    from concourse.bass_utils import run_bass_kernel_spmd

    ensure_ntff_hook()
    nc = get_bass()
    in_maps = shard_input(output_spikes)
    res = run_bass_kernel_spmd(nc, in_maps, core_ids=list(range(NCORES)))
    acc_list = [res.results[c]["acc"] for c in range(NCORES)]
    return finish_host(acc_list, target_cv)


# revision 3
# speedup vs baseline: 1.0771x; 1.0771x over previous
"""v4: age-scan CVLoss kernel.

Key identity: with age_t = distance since last spike (0 at a spike) and
M_t = t - age_t (last-spike position, chained across chunks),
  sum of ISI^2 (incl. phantom 0->f gap) = l^2 - 2*(S_M - (F-l+1)*l)
  where S_M = F(F+1)/2 - sum(age).
So the device only needs, per row: per-chunk sums of y=1-x (-> k), sums of
age, the final age (-> l), and a tiny reverse scan on chunks 0/1 (-> f).

Per chunk (W=2000):
  ACT : y = Copy(-x+1)  f32->i16, accum -> sum(y)   [k_c = W - sum]
  DVE : age = ttscan(y, y, op0=mult, op1=add, initial=prev)  (chained)
  ACT : Copy(age) accum -> sum(age)
  DVE : chunks 0,1 only: reverse ttscan -> r1 (distance to first spike)
Host: stitch 2 halves per neuron, compute CV and nan-mean loss.
"""

import numpy as np

B, T, N = 16, 2000, 512
L = B * T
NCORES = 8
NPC = N // NCORES
HALVES = 2
P = NPC * HALVES
F = L // HALVES
W = 4000
NCH = F // W
NACC = 2 * NCH + 3  # ky[8] | sage[8] | age_last | r1_c0 | r1_c1

_BUILD_CACHE = {}


def build_bass(F_=F, W_=W, P_=P):
    import concourse.bass as bass
    from concourse import bacc
    import concourse.mybir as mybir
    from concourse import tile

    nch = F_ // W_
    Alu = mybir.AluOpType
    AF = mybir.ActivationFunctionType
    f32 = mybir.dt.float32
    i16 = mybir.dt.int16
    bf16 = mybir.dt.bfloat16

    nc = bacc.Bacc(trn_type="TRN2")
    x = nc.dram_tensor("x", (P_, F_), f32, kind="ExternalInput")
    acc = nc.dram_tensor("acc", (P_, NACC), f32, kind="ExternalOutput")

    with tile.TileContext(nc) as tc:
        with tc.tile_pool(name="persist", bufs=1) as pp, \
             tc.tile_pool(name="work", bufs=3) as wp:
            accs = pp.tile([P_, NACC], f32)

            def load_pass1(c):
                lo = c * W_
                xc = wp.tile([P_, W_], f32, tag="xc", name=f"xc{c}")
                nc.sync.dma_start(out=xc[:], in_=x[:, lo:lo + W_])
                yc = wp.tile([P_, W_], i16, tag="y", name=f"y{c}")
                nc.scalar.activation(
                    out=yc[:], in_=xc[:], func=AF.Copy,
                    scale=-1.0, bias=1.0,
                    accum_out=accs[:, c:c + 1])
                return yc

            def scan(c, yc, prev_age):
                age_c = wp.tile([P_, W_], i16, tag="age", name=f"age{c}")
                init = 0.0 if prev_age is None else prev_age[:, W_ - 1:W_]
                nc.vector.tensor_tensor_scan(
                    out=age_c[:], data0=yc[:], data1=yc[:], initial=init,
                    op0=Alu.mult, op1=Alu.add)
                return age_c

            def rev_scan(c, yc):
                rev = wp.tile([P_, W_], i16, tag="rev", name=f"rev{c}")
                nc.vector.tensor_tensor_scan(
                    out=rev[:, ::-1], data0=yc[:, ::-1], data1=yc[:, ::-1],
                    initial=0.0, op0=Alu.mult, op1=Alu.add)
                nc.vector.tensor_scalar(
                    out=accs[:, 2 * nch + 1 + c:2 * nch + 2 + c],
                    in0=rev[:, 0:1], scalar1=0.0, scalar2=None, op0=Alu.add)

            def sum_age(c, age_c):
                dummy = wp.tile([P_, W_], bf16, tag="dummy", name=f"dm{c}")
                nc.scalar.activation(
                    out=dummy[:], in_=age_c[:], func=AF.Copy,
                    accum_out=accs[:, nch + c:nch + c + 1])

            prev_age = None
            pending = None
            for c in range(nch):
                yc = load_pass1(c)
                if pending is not None:
                    sum_age(*pending)
                age_c = scan(c, yc, prev_age)
                if c < 2:
                    rev_scan(c, yc)
                pending = (c, age_c)
                prev_age = age_c
            sum_age(*pending)
            nc.vector.tensor_scalar(
                out=accs[:, 2 * nch:2 * nch + 1],
                in0=prev_age[:, W_ - 1:W_], scalar1=0.0, scalar2=None,
                op0=Alu.add)

            nc.sync.dma_start(out=acc[:], in_=accs[:])
    nc.finalize()
    return nc


def get_bass():
    key = (F, W, P)
    if key not in _BUILD_CACHE:
        _BUILD_CACHE[key] = build_bass()
    return _BUILD_CACHE[key]


def shard_input(output_spikes):
    x = np.asarray(output_spikes, dtype=np.float32)
    maps = []
    for c in range(NCORES):
        xc = x[:, :, c * NPC:(c + 1) * NPC]
        xt = np.ascontiguousarray(np.transpose(xc, (2, 0, 1))).reshape(NPC, L)
        maps.append({"x": xt.reshape(P, F)})
    return maps


def finish_host(acc_list, target_cv, F_=F, W_=W, nch=NCH):
    """Merge per-row device stats into the scalar loss (float64)."""
    target = np.asarray(target_cv, dtype=np.float64)
    sq_sum = 0.0
    n_valid = 0
    for ci, a in enumerate(acc_list):
        a = np.asarray(a, dtype=np.float64)
        ky = a[:, 0:nch]
        sage = a[:, nch:2 * nch]
        age_last = a[:, 2 * nch]
        r1 = a[:, 2 * nch + 1:2 * nch + 3]
        k_c = W_ - ky
        k = k_c.sum(axis=1)
        S_age = sage.sum(axis=1)
        ll = F_ - age_last
        S_M = F_ * (F_ + 1) / 2.0 - S_age
        s2_ph = ll * ll - 2.0 * (S_M - (F_ - ll + 1.0) * ll)
        f = np.where(k_c[:, 0] > 0, r1[:, 0] + 1.0,
                     np.where(k_c[:, 1] > 0, W_ + r1[:, 1] + 1.0, 1.0))
        s2 = s2_ph - f * f
        n_neu = a.shape[0] // 2
        for n in range(n_neu):
            p0, p1 = 2 * n, 2 * n + 1
            k0, k1 = k[p0], k[p1]
            kt = k0 + k1
            if kt < 3:
                continue
            if k0 > 0 and k1 > 0:
                s2t = s2[p0] + s2[p1] + (F_ + f[p1] - ll[p0]) ** 2
                s1t = F_ + ll[p1] - f[p0]
            elif k0 > 0:
                s2t, s1t = s2[p0], ll[p0] - f[p0]
            else:
                s2t, s1t = s2[p1], ll[p1] - f[p1]
            mean = s1t / (kt - 1.0)
            var = (s2t - s1t * s1t / (kt - 1.0)) / (kt - 2.0)
            std = np.sqrt(var) if var > 0 else 0.0
            if mean <= 0:
                continue
            cv = std / max(mean, 1e-12)
            d = cv - target[ci * NPC + n]
            sq_sum += d * d
            n_valid += 1
    return np.float32(sq_sum / max(n_valid, 1))


def ensure_ntff_hook(so_path="/opt/axon/libaxon_pjrt.so"):
    """Shim antenv.axon_hooks (absent in this image) so trace=True works.

    Mirrors trn_boot._ntff_profile_via_ctypes: drives NRT profiling via the
    axon PJRT .so's C ABI. Safe no-op if anything is missing.
    """
    import sys
    try:
        import antenv.axon_hooks  # noqa: F401
        return
    except ImportError:
        pass
    try:
        import ctypes
        import contextlib
        import types
        import os

        if not os.path.exists(so_path):
            return
        lib = ctypes.CDLL(so_path)
        if not hasattr(lib, "axon_start_nrt_profile"):
            return
        lib.axon_start_nrt_profile.argtypes = [
            ctypes.POINTER(ctypes.c_int64), ctypes.c_size_t]
        lib.axon_start_nrt_profile.restype = ctypes.c_int64
        lib.axon_stop_nrt_profile.argtypes = [ctypes.c_char_p]
        lib.axon_stop_nrt_profile.restype = ctypes.c_int64

        @contextlib.contextmanager
        def _hook(output_dir, device_ids):
            import jax
            jax.devices()
            if device_ids:
                ids = (ctypes.c_int64 * len(device_ids))(*device_ids)
                rc = lib.axon_start_nrt_profile(ids, len(device_ids))
            else:
                rc = lib.axon_start_nrt_profile(None, 0)
            if rc != 0:
                raise RuntimeError(f"axon_start_nrt_profile rc={rc}")
            try:
                yield
            finally:
                n = lib.axon_stop_nrt_profile(str(output_dir).encode())
                print(f"profile: {n} file(s) written to {output_dir}",
                      file=sys.stderr)

        mod = types.ModuleType("antenv.axon_hooks")
        mod.get_axon_ntff_profile_hook = lambda: _hook
        mod.set_axon_ntff_profile_hook = lambda h: None
        import antenv
        sys.modules["antenv.axon_hooks"] = mod
        antenv.axon_hooks = mod
    except Exception:
        pass


def kernel(output_spikes, target_cv):
    from concourse.bass_utils import run_bass_kernel_spmd

    ensure_ntff_hook()
    nc = get_bass()
    in_maps = shard_input(output_spikes)
    res = run_bass_kernel_spmd(nc, in_maps, core_ids=list(range(NCORES)))
    acc_list = [res.results[c]["acc"] for c in range(NCORES)]
    return finish_host(acc_list, target_cv)
